# revision 44
# baseline (speedup 1.0000x reference)
"""AlignUniform loss kernel for Trainium2 (8 NeuronCores, SPMD).

Math:
  qn = q / ||q||, kn = k / ||k||         (row-wise L2 normalize)
  align = mean_i ||qn_i - kn_i||^2
  lunif(x) = log( sum_{i<j} exp(-2*||x_i-x_j||^2) / npairs )
           = log( sum_{i<j} exp(4*<x_i,x_j> - 4) / npairs )   (unit-norm rows)
  out = align + (lunif(qn) + lunif(kn)) / 2

Sharding: the strict-upper pairwise sum is decomposed into 512x512 blocks of
the NxN gram matrix.  With 16 row-blocks, there are 16 diagonal blocks and 120
unordered off-diagonal block pairs; each unordered pair {a,b} is covered
exactly once by the rotation pairs (b, b+r mod 16) for r=1..7 plus the 8 pairs
(c, c+8).  Each of the 8 cores gets a uniform slice: 2 diagonal blocks +
15 off-diagonal pairs = 17 units of [512, 512].  Per-core inputs are
host-gathered so the compiled program is identical on every core (SPMD), and
the per-unit exp-sums come back as [128]-vectors that the host folds into the
final scalar (the "all-reduce before log" step).

Device pipeline per core: DMA gathered rows (fp32) -> row sumsq (GpSimd
square + DVE reduce) -> rsqrt (ACT sqrt + DVE reciprocal) -> scale rows with
fused bf16 cast (DVE) -> transpose to [D, rows] layout via DMA-XBAR (bf16) ->
gram matmuls (PE, bf16 in / fp32 PSUM accum) -> exp(4s-4) + free-axis reduce
(ACT, one instruction per 4-bank PSUM unit) -> tiny accumulator DMA out.
bf16 rounding of the *normalized unit vectors* is safe here: the final error
after the 33M-element exp-sum measures ~1e-6 relative (rounding errors are
zero-mean and average out); align is computed from fp32 values.
"""

import functools

import numpy as np

import concourse.bacc as bacc
import concourse.mybir as mybir
import concourse.tile as tile

# ----------------------------------------------------------------------------
# Problem constants (hardcoded per harness contract).
N = 8192
D = 128
NCORES = 8
NB = 16           # row blocks
BLK = N // NB     # 512
NSLOT = 11        # gathered blocks per core (slots 0..10)
GROWS = NSLOT * BLK   # 5632 gathered rows per core per tensor
NT = GROWS // 128     # 44 natural [128, D] tiles
CH = 4                # tiles per chunk (= one 512-row slot)
NCH = NT // CH        # 11 chunks == slots

# unit list: (row_slot, col_slot, is_diag) -- identical on every core.
UNITS = (
    [(0, 0, True), (1, 1, True)]
    + [(0, r, False) for r in range(1, 8)]
    + [(1, 1 + r, False) for r in range(1, 8)]
    + [(10, 9, False)]
)
NU = len(UNITS)  # 17
NACC = NU + 4  # unit cols + 4 piece-cols for the split first unit (diag)

MM_DT = mybir.dt.bfloat16  # gram matmul operand dtype

ACC_COLS = 64  # output: [0:21) q unit cols, [21:42) k unit cols, [42:50) align


def _core_blocks(c: int) -> list[int]:
    """Row-block indices gathered for core c, slot order 0..10."""
    return [(2 * c + s) % NB for s in range(9)] + [(c + 8) % NB, c]


# ----------------------------------------------------------------------------
# Workaround: this walrus build rejects >1 semaphore wait per instruction, but
# TileContext's stock exit drain carries one wait per active proc.  Split it
# into one single-wait drain per proc.
def _apply_tile_exit_patch():
    import re

    import bass_rust
    from concourse.vector_clock import ScopedClock

    if getattr(tile.TileContext, "_drain_split_patch", False):
        return

    def _drain_and_barrier(self, tick_clock, wait_clock):
        nc = self.nc
        ticks = [int(s) for s in re.findall(r"\d+", repr(tick_clock.global_clock))]
        for p, t in ((p, t) for p, t in enumerate(ticks) if t > 0):
            vc = bass_rust.VectorClock()
            vc.require_at_least(p, t)
            d = nc.sync.drain()
            wait_clock.add_sem_waits(d.ins, ScopedClock({None: vc}))
        nc.all_engine_barrier()
        assert self.sems is not None
        popped = nc._tile_sem_poison_stack.pop()
        assert popped is self._sem_poison
        nc.clear_and_free_semaphores(list(self.sems.allocated().values()))
        nc.all_engine_barrier()

    tile.TileContext._drain_and_barrier = _drain_and_barrier
    tile.TileContext._drain_split_patch = True


def _apply_act_table_patch():
    """Prefer the table set containing BOTH Ln and Exp so the whole kernel
    runs on a single ACT table load (Ln alone resolves to `natural_log`, Exp
    to `exp_and_others`, and alternating them reloads tables at 1.3us each)."""
    import concourse.hw_specs as hw_specs

    orig = hw_specs.get_activation_tables
    if getattr(orig, "_pref_patch", False):
        return

    def patched(arch):
        t = orig(arch)
        pref = "natural_log_exp_and_others"
        if pref not in t:
            return t
        AF = mybir.ActivationFunctionType
        out = {}
        for k, fns in t.items():  # keep order: set ids index into act_info.json
            if k != pref:
                fns = set(fns) - {AF.Exp, AF.Ln}
            out[k] = fns
        return out

    patched._pref_patch = True
    hw_specs.get_activation_tables = patched
    bacc.get_activation_tables = patched


# ----------------------------------------------------------------------------
GROUPS = [(0, 1), (1, 2), (2, 6), (6, 11)]  # slot ranges: fast path first


def _emit(nc, tc, ctx, ins_dram, out_dram):
    f32 = mybir.dt.float32
    AF = mybir.ActivationFunctionType
    ALU = mybir.AluOpType

    big = ctx.enter_context(tc.tile_pool(name="big", bufs=1))
    scratch = ctx.enter_context(tc.tile_pool(name="scratch", bufs=2))
    dump = ctx.enter_context(tc.tile_pool(name="dump", bufs=1))
    psp = ctx.enter_context(tc.tile_pool(name="ps", bufs=2, space="PSUM"))

    # persistent buffers: natf[ti][g] holds slots GROUPS[g] in natural fp32
    natf = [
        [
            big.tile([128, (g1 - g0) * CH, D], f32, tag=f"natf{ti}_{g}", name=f"natf{ti}_{g}")
            for g, (g0, g1) in enumerate(GROUPS)
        ]
        for ti in range(2)
    ]
    qts = [
        [big.tile([128, BLK], MM_DT, tag=f"qt{ti}_{s}", name=f"qt{ti}_{s}") for s in range(NSLOT)]
        for ti in range(2)
    ]
    accs = [big.tile([128, NACC], f32, tag=f"acc{ti}", name=f"acc{ti}") for ti in range(2)]
    for ti in range(2):
        nc.vector.memset(accs[ti][:, 0:1], 0.0)  # unit 0 reported via piece cols
    rns = [big.tile([128, NT], f32, tag=f"rn{ti}", name=f"rn{ti}") for ti in range(2)]
    ssqs = [big.tile([128, NT], f32, tag=f"ssq{ti}", name=f"ssq{ti}") for ti in range(2)]
    acc_align = big.tile([128, 8], f32, tag="accalign")
    biasm4 = big.tile([128, 1], f32, tag="biasm4")
    nc.vector.memset(biasm4, -4.0)
    u32 = mybir.dt.uint32
    magic = big.tile([128, 1], u32, tag="magic")
    nc.vector.memset(magic, 0x5F3759DF)

    def dma_group(ti, g):
        g0, g1 = GROUPS[g]
        src = ins_dram[ti].rearrange("(t p) d -> p t d", p=128)
        nc.sync.dma_start(natf[ti][g][:], src[:, CH * g0 : CH * g1, :])

    def sumsq_group(ti, g, square_engine):
        """Square + row-reduce for slots GROUPS[g] of tensor ti."""
        g0, g1 = GROUPS[g]
        nt = (g1 - g0) * CH
        nf = natf[ti][g]
        sq = scratch.tile([128, nt, D], f32, tag=f"sq_scratch{g}", name=f"sq{ti}_{g}")
        square_engine.tensor_tensor(sq[:], nf[:], nf[:], ALU.mult)
        nc.vector.tensor_reduce(
            ssqs[ti][:, CH * g0 : CH * g1], sq[:], mybir.AxisListType.X, ALU.add
        )

    def rnorm_group(ti, g):
        """rn = 1/sqrt(ssq) via magic-constant + 2 Newton iterations, all on
        DVE -- keeps ScalarE exclusively on Exp (single table set)."""
        g0, g1 = GROUPS[g]
        nt = g1 - g0
        sl = slice(CH * g0, CH * g1)
        x = ssqs[ti][:, sl]
        y = rns[ti][:, sl]
        yu = y.bitcast(u32)
        hx = scratch.tile([128, CH * nt], f32, tag="nr_hx")
        tmp = scratch.tile([128, CH * nt], f32, tag="nr_tmp")
        nc.vector.tensor_scalar(yu, x.bitcast(u32), 1, None, op0=ALU.logical_shift_right)
        nc.vector.tensor_tensor(yu, magic[:, 0:1].to_broadcast((128, CH * nt)), yu, ALU.subtract)
        nc.vector.tensor_scalar(hx[:], x, 0.5, None, op0=ALU.mult)
        for _ in range(2):
            nc.vector.tensor_tensor(tmp[:], y, y, ALU.mult)
            nc.vector.tensor_tensor(tmp[:], tmp[:], hx[:], ALU.mult)
            nc.vector.tensor_scalar(tmp[:], tmp[:], -1.0, 1.5, op0=ALU.mult, op1=ALU.add)
            nc.vector.tensor_tensor(y, y, tmp[:], ALU.mult)

    def apply_transpose_group(ti, g):
        """nat2 = natf * rn (fp32 TT + bf16 cast copy), then per-slot XBAR transpose."""
        g0, g1 = GROUPS[g]
        nt = (g1 - g0) * CH
        nf = natf[ti][g]
        napp = scratch.tile([128, nt, D], f32, tag=f"napp_scratch{g}", name=f"na{ti}_{g}")
        n2 = scratch.tile([128, nt, D], MM_DT, tag=f"nat2_scratch{g}", name=f"n2{ti}_{g}")
        rnb = rns[ti][:, CH * g0 : CH * g1, None].to_broadcast((128, nt, D))
        nc.vector.tensor_tensor(napp[:], nf[:], rnb, ALU.mult)
        nc.vector.tensor_copy(out=n2[:], in_=napp[:])
        for s in range(g0, g1):
            qt3 = qts[ti][s].rearrange("d (t p) -> d t p", p=128)
            nc.sync.dma_start_transpose(
                qt3[:, :, :], n2[:, CH * (s - g0) : CH * (s - g0 + 1), :].rearrange("p t d -> p (t d)")
            )

    # ---- units: 4 gram matmuls into a 4-bank PSUM tile + one exp-reduce ----
    def emit_unit(ti, u, split=False):
        rs, cs, _ = UNITS[u]
        ps = psp.tile([128, 2048], f32, tag="ps", name=f"ps{ti}_{u}")
        expd = dump.tile([128, 2048], f32, tag="expdump")
        for m in range(4):
            nc.tensor.matmul(
                ps[:, 512 * m : 512 * (m + 1)],
                lhsT=qts[ti][rs][:, 128 * m : 128 * (m + 1)],
                rhs=qts[ti][cs][:],
                start=True,
                stop=True,
            )
            if split:  # one exp per matmul: shortens the pipeline lead-in
                nc.scalar.activation(
                    expd[:, 512 * m : 512 * (m + 1)],
                    ps[:, 512 * m : 512 * (m + 1)],
                    AF.Exp,
                    bias=biasm4[:],
                    scale=4.0,
                    accum_out=accs[ti][:, NU + m : NU + m + 1],
                )
        if not split:
            nc.scalar.activation(
                expd[:],
                ps[:],
                AF.Exp,
                bias=biasm4[:],
                scale=4.0,
                accum_out=accs[ti][:, u : u + 1],
            )

    # unit waves by the largest slot they touch (group boundary)
    def wave(g):
        lo = -1 if g == 0 else GROUPS[g - 1][1] - 1
        hi = GROUPS[g][1] - 1
        return [u for u, (rs, cs, _) in enumerate(UNITS) if lo < max(rs, cs) <= hi]

    # ---- emission: fast path (slot 0), later groups pipelined behind waves
    for ti in range(2):
        dma_group(ti, 0)
    for ti in range(2):
        sumsq_group(ti, 0, nc.vector if ti == 0 else nc.gpsimd)
        rnorm_group(ti, 0)
        apply_transpose_group(ti, 0)
    for ti in range(2):
        dma_group(ti, 1)
        sumsq_group(ti, 1, nc.vector if ti == 0 else nc.gpsimd)
    for u in wave(0):
        for ti in range(2):
            emit_unit(ti, u, split=True)
    for ti in range(2):
        rnorm_group(ti, 1)
        apply_transpose_group(ti, 1)
    for ti in range(2):
        dma_group(ti, 2)
        sumsq_group(ti, 2, nc.gpsimd if ti == 0 else nc.vector)
    for u in wave(1):
        for ti in range(2):
            emit_unit(ti, u)
    for ti in range(2):
        rnorm_group(ti, 2)
        apply_transpose_group(ti, 2)
    for ti in range(2):
        dma_group(ti, 3)
        sumsq_group(ti, 3, nc.gpsimd if ti == 0 else nc.vector)
    for u in wave(2):
        for ti in range(2):
            emit_unit(ti, u)
    for ti in range(2):
        rnorm_group(ti, 3)
        apply_transpose_group(ti, 3)

    # ---- align term from fp32 groups 0,1 (slots 0,1 = all N rows once) ----
    for g in range(2):
        qn = scratch.tile([128, CH, D], f32, tag="align_q")
        kn = scratch.tile([128, CH, D], f32, tag="align_k")
        sl = slice(CH * g, CH * (g + 1))
        rq = rns[0][:, sl, None].to_broadcast((128, CH, D))
        rk = rns[1][:, sl, None].to_broadcast((128, CH, D))
        nc.vector.tensor_tensor(qn[:], natf[0][g][:], rq, ALU.mult)
        nc.vector.tensor_tensor(kn[:], natf[1][g][:], rk, ALU.mult)
        nc.vector.tensor_tensor(qn[:], qn[:], kn[:], ALU.subtract)
        nc.gpsimd.tensor_tensor(qn[:], qn[:], qn[:], ALU.mult)
        nc.vector.tensor_reduce(acc_align[:, sl], qn[:], mybir.AxisListType.X, ALU.add)

    for u in wave(3):
        for ti in range(2):
            emit_unit(ti, u)

    # ---- write accumulators out
    nc.sync.dma_start(out_dram[:, 0:NACC], accs[0][:])
    nc.sync.dma_start(out_dram[:, NACC : 2 * NACC], accs[1][:])
    nc.sync.dma_start(out_dram[:, 2 * NACC : 2 * NACC + 8], acc_align[:])


@functools.lru_cache(maxsize=1)
def _build():
    from contextlib import ExitStack

    _apply_tile_exit_patch()
    nc = bacc.Bacc("TRN2", target_bir_lowering=False, debug=False, num_devices=NCORES)
    f32 = mybir.dt.float32
    qg = nc.dram_tensor("qg", [GROWS, D], f32, kind="ExternalInput")
    kg = nc.dram_tensor("kg", [GROWS, D], f32, kind="ExternalInput")
    out = nc.dram_tensor("out", [128, ACC_COLS], f32, kind="ExternalOutput")
    with tile.TileContext(nc) as tc, ExitStack() as ctx:
        _emit(nc, tc, ctx, (qg.ap(), kg.ap()), out.ap())
    nc.compile()
    return nc


def _gather(x: np.ndarray, c: int) -> np.ndarray:
    return np.ascontiguousarray(
        np.concatenate([x[BLK * b : BLK * (b + 1)] for b in _core_blocks(c)])
    )


def run_device(q: np.ndarray, k: np.ndarray, **run_kwargs):
    """Compile + run on the 8 cores; returns BassKernelResults."""
    from concourse.bass_utils import run_bass_kernel_spmd

    nc = _build()
    in_maps = [{"qg": _gather(q, c), "kg": _gather(k, c)} for c in range(NCORES)]
    return run_bass_kernel_spmd(nc, in_maps, core_ids=list(range(NCORES)), **run_kwargs)


def reduce_outputs(outs: list) -> np.float32:
    """Host-side gather/unshard: fold per-core accumulators into the scalar."""
    npairs = N * (N - 1) / 2.0
    terms = []
    for ti in range(2):
        off = 0.0
        diag = 0.0
        for c in range(NCORES):
            sums = outs[c]["out"][:, ti * NACC : (ti + 1) * NACC].astype(np.float64).sum(axis=0)
            for u, (_, _, is_diag) in enumerate(UNITS):
                if is_diag:
                    diag += sums[u]
                else:
                    off += sums[u]
            diag += sums[NU : NU + 4].sum()  # split unit-0 pieces (diag unit)
        upper = off + (diag - N) / 2.0
        terms.append(np.log(upper / npairs))
    align = (
        sum(
            outs[c]["out"][:, 2 * NACC : 2 * NACC + 8].astype(np.float64).sum()
            for c in range(NCORES)
        )
        / N
    )
    return np.float32(align + (terms[0] + terms[1]) / 2.0)


def kernel(q: np.ndarray, k: np.ndarray) -> np.ndarray:
    res = run_device(q, k)
    return np.asarray(reduce_outputs(res.results), dtype=np.float32)


# revision 45
# speedup vs baseline: 1.0064x; 1.0064x over previous
"""AlignUniform loss kernel for Trainium2 (8 NeuronCores, SPMD).

Math:
  qn = q / ||q||, kn = k / ||k||         (row-wise L2 normalize)
  align = mean_i ||qn_i - kn_i||^2
  lunif(x) = log( sum_{i<j} exp(-2*||x_i-x_j||^2) / npairs )
           = log( sum_{i<j} exp(4*<x_i,x_j> - 4) / npairs )   (unit-norm rows)
  out = align + (lunif(qn) + lunif(kn)) / 2

Sharding: the strict-upper pairwise sum is decomposed into 512x512 blocks of
the NxN gram matrix.  With 16 row-blocks, there are 16 diagonal blocks and 120
unordered off-diagonal block pairs; each unordered pair {a,b} is covered
exactly once by the rotation pairs (b, b+r mod 16) for r=1..7 plus the 8 pairs
(c, c+8).  Each of the 8 cores gets a uniform slice: 2 diagonal blocks +
15 off-diagonal pairs = 17 units of [512, 512].  Per-core inputs are
host-gathered so the compiled program is identical on every core (SPMD), and
the per-unit exp-sums come back as [128]-vectors that the host folds into the
final scalar (the "all-reduce before log" step).

Device pipeline per core: DMA gathered rows (fp32) -> row sumsq (GpSimd
square + DVE reduce) -> rsqrt (ACT sqrt + DVE reciprocal) -> scale rows with
fused bf16 cast (DVE) -> transpose to [D, rows] layout via DMA-XBAR (bf16) ->
gram matmuls (PE, bf16 in / fp32 PSUM accum) -> exp(4s-4) + free-axis reduce
(ACT, one instruction per 4-bank PSUM unit) -> tiny accumulator DMA out.
bf16 rounding of the *normalized unit vectors* is safe here: the final error
after the 33M-element exp-sum measures ~1e-6 relative (rounding errors are
zero-mean and average out); align is computed from fp32 values.
"""

import functools

import numpy as np

import concourse.bacc as bacc
import concourse.mybir as mybir
import concourse.tile as tile

# ----------------------------------------------------------------------------
# Problem constants (hardcoded per harness contract).
N = 8192
D = 128
NCORES = 8
NB = 16           # row blocks
BLK = N // NB     # 512
NSLOT = 11        # gathered blocks per core (slots 0..10)
GROWS = NSLOT * BLK   # 5632 gathered rows per core per tensor
NT = GROWS // 128     # 44 natural [128, D] tiles
CH = 4                # tiles per chunk (= one 512-row slot)
NCH = NT // CH        # 11 chunks == slots

# unit list: (row_slot, col_slot, is_diag) -- identical on every core.
UNITS = (
    [(0, 0, True), (1, 1, True)]
    + [(0, r, False) for r in range(1, 8)]
    + [(1, 1 + r, False) for r in range(1, 8)]
    + [(10, 9, False)]
)
NU = len(UNITS)  # 17
NACC = NU + 4  # unit cols + 4 piece-cols for the split first unit (diag)

MM_DT = mybir.dt.bfloat16  # gram matmul operand dtype

ACC_COLS = 64  # output: [0:21) q unit cols, [21:42) k unit cols, [42:50) align


def _core_blocks(c: int) -> list[int]:
    """Row-block indices gathered for core c, slot order 0..10."""
    return [(2 * c + s) % NB for s in range(9)] + [(c + 8) % NB, c]


# ----------------------------------------------------------------------------
# Workaround: this walrus build rejects >1 semaphore wait per instruction, but
# TileContext's stock exit drain carries one wait per active proc.  Split it
# into one single-wait drain per proc.
def _apply_tile_exit_patch():
    import re

    import bass_rust
    from concourse.vector_clock import ScopedClock

    if getattr(tile.TileContext, "_drain_split_patch", False):
        return

    def _drain_and_barrier(self, tick_clock, wait_clock):
        nc = self.nc
        ticks = [int(s) for s in re.findall(r"\d+", repr(tick_clock.global_clock))]
        for p, t in ((p, t) for p, t in enumerate(ticks) if t > 0):
            vc = bass_rust.VectorClock()
            vc.require_at_least(p, t)
            d = nc.sync.drain()
            wait_clock.add_sem_waits(d.ins, ScopedClock({None: vc}))
        nc.all_engine_barrier()
        assert self.sems is not None
        popped = nc._tile_sem_poison_stack.pop()
        assert popped is self._sem_poison
        nc.clear_and_free_semaphores(list(self.sems.allocated().values()))
        nc.all_engine_barrier()

    tile.TileContext._drain_and_barrier = _drain_and_barrier
    tile.TileContext._drain_split_patch = True


def _apply_act_table_patch():
    """Prefer the table set containing BOTH Ln and Exp so the whole kernel
    runs on a single ACT table load (Ln alone resolves to `natural_log`, Exp
    to `exp_and_others`, and alternating them reloads tables at 1.3us each)."""
    import concourse.hw_specs as hw_specs

    orig = hw_specs.get_activation_tables
    if getattr(orig, "_pref_patch", False):
        return

    def patched(arch):
        t = orig(arch)
        pref = "natural_log_exp_and_others"
        if pref not in t:
            return t
        AF = mybir.ActivationFunctionType
        out = {}
        for k, fns in t.items():  # keep order: set ids index into act_info.json
            if k != pref:
                fns = set(fns) - {AF.Exp, AF.Ln}
            out[k] = fns
        return out

    patched._pref_patch = True
    hw_specs.get_activation_tables = patched
    bacc.get_activation_tables = patched


# ----------------------------------------------------------------------------
GROUPS = [(0, 1), (1, 2), (2, 6), (6, 11)]  # slot ranges: fast path first


def _emit(nc, tc, ctx, ins_dram, out_dram):
    f32 = mybir.dt.float32
    AF = mybir.ActivationFunctionType
    ALU = mybir.AluOpType

    big = ctx.enter_context(tc.tile_pool(name="big", bufs=1))
    scratch = ctx.enter_context(tc.tile_pool(name="scratch", bufs=2))
    dump = ctx.enter_context(tc.tile_pool(name="dump", bufs=1))
    psp = ctx.enter_context(tc.tile_pool(name="ps", bufs=2, space="PSUM"))

    # persistent buffers: natf[ti][g] holds slots GROUPS[g] in natural fp32
    natf = [
        [
            big.tile([128, (g1 - g0) * CH, D], f32, tag=f"natf{ti}_{g}", name=f"natf{ti}_{g}")
            for g, (g0, g1) in enumerate(GROUPS)
        ]
        for ti in range(2)
    ]
    qts = [
        [big.tile([128, BLK], MM_DT, tag=f"qt{ti}_{s}", name=f"qt{ti}_{s}") for s in range(NSLOT)]
        for ti in range(2)
    ]
    accs = [big.tile([128, NACC], f32, tag=f"acc{ti}", name=f"acc{ti}") for ti in range(2)]
    for ti in range(2):
        nc.vector.memset(accs[ti][:, 0:1], 0.0)  # unit 0 reported via piece cols
    rns = [big.tile([128, NT], f32, tag=f"rn{ti}", name=f"rn{ti}") for ti in range(2)]
    ssqs = [big.tile([128, NT], f32, tag=f"ssq{ti}", name=f"ssq{ti}") for ti in range(2)]
    acc_align = big.tile([128, 8], f32, tag="accalign")
    biasm4 = big.tile([128, 1], f32, tag="biasm4")
    nc.vector.memset(biasm4, -4.0)
    u32 = mybir.dt.uint32
    magic = big.tile([128, 1], u32, tag="magic")
    nc.vector.memset(magic, 0x5F3759DF)

    def dma_group(ti, g):
        g0, g1 = GROUPS[g]
        src = ins_dram[ti].rearrange("(t p) d -> p t d", p=128)
        nc.sync.dma_start(natf[ti][g][:], src[:, CH * g0 : CH * g1, :])

    def sumsq_group(ti, g, square_engine):
        """Square + row-reduce for slots GROUPS[g] of tensor ti."""
        g0, g1 = GROUPS[g]
        nt = (g1 - g0) * CH
        nf = natf[ti][g]
        sq = scratch.tile([128, nt, D], f32, tag=f"sq_scratch{g}", name=f"sq{ti}_{g}")
        square_engine.tensor_tensor(sq[:], nf[:], nf[:], ALU.mult)
        nc.vector.tensor_reduce(
            ssqs[ti][:, CH * g0 : CH * g1], sq[:], mybir.AxisListType.X, ALU.add
        )

    def rnorm_group(ti, g):
        """rn = 1/sqrt(ssq) via magic-constant + 2 Newton iterations, all on
        DVE -- keeps ScalarE exclusively on Exp (single table set)."""
        g0, g1 = GROUPS[g]
        nt = g1 - g0
        sl = slice(CH * g0, CH * g1)
        x = ssqs[ti][:, sl]
        y = rns[ti][:, sl]
        yu = y.bitcast(u32)
        hx = scratch.tile([128, CH * nt], f32, tag="nr_hx")
        tmp = scratch.tile([128, CH * nt], f32, tag="nr_tmp")
        nc.vector.tensor_scalar(yu, x.bitcast(u32), 1, None, op0=ALU.logical_shift_right)
        nc.vector.tensor_tensor(yu, magic[:, 0:1].to_broadcast((128, CH * nt)), yu, ALU.subtract)
        nc.vector.tensor_scalar(hx[:], x, 0.5, None, op0=ALU.mult)
        for _ in range(2):
            nc.vector.tensor_tensor(tmp[:], y, y, ALU.mult)
            nc.vector.tensor_tensor(tmp[:], tmp[:], hx[:], ALU.mult)
            nc.vector.tensor_scalar(tmp[:], tmp[:], -1.0, 1.5, op0=ALU.mult, op1=ALU.add)
            nc.vector.tensor_tensor(y, y, tmp[:], ALU.mult)

    def apply_transpose_group(ti, g):
        """nat2 = natf * rn (GpSimd, bf16 cast on write), then per-slot XBAR
        transpose.  Keeps DVE off the first-unit critical path."""
        g0, g1 = GROUPS[g]
        nt = (g1 - g0) * CH
        nf = natf[ti][g]
        n2 = scratch.tile([128, nt, D], MM_DT, tag=f"nat2_scratch{g}", name=f"n2{ti}_{g}")
        rnb = rns[ti][:, CH * g0 : CH * g1, None].to_broadcast((128, nt, D))
        nc.gpsimd.tensor_tensor(n2[:], nf[:], rnb, ALU.mult)
        for s in range(g0, g1):
            qt3 = qts[ti][s].rearrange("d (t p) -> d t p", p=128)
            nc.sync.dma_start_transpose(
                qt3[:, :, :], n2[:, CH * (s - g0) : CH * (s - g0 + 1), :].rearrange("p t d -> p (t d)")
            )

    # ---- units: 4 gram matmuls into a 4-bank PSUM tile + one exp-reduce ----
    def emit_unit(ti, u, split=False):
        rs, cs, _ = UNITS[u]
        ps = psp.tile([128, 2048], f32, tag="ps", name=f"ps{ti}_{u}")
        expd = dump.tile([128, 2048], f32, tag="expdump")
        for m in range(4):
            nc.tensor.matmul(
                ps[:, 512 * m : 512 * (m + 1)],
                lhsT=qts[ti][rs][:, 128 * m : 128 * (m + 1)],
                rhs=qts[ti][cs][:],
                start=True,
                stop=True,
            )
            if split:  # one exp per matmul: shortens the pipeline lead-in
                nc.scalar.activation(
                    expd[:, 512 * m : 512 * (m + 1)],
                    ps[:, 512 * m : 512 * (m + 1)],
                    AF.Exp,
                    bias=biasm4[:],
                    scale=4.0,
                    accum_out=accs[ti][:, NU + m : NU + m + 1],
                )
        if not split:
            nc.scalar.activation(
                expd[:],
                ps[:],
                AF.Exp,
                bias=biasm4[:],
                scale=4.0,
                accum_out=accs[ti][:, u : u + 1],
            )

    # unit waves by the largest slot they touch (group boundary)
    def wave(g):
        lo = -1 if g == 0 else GROUPS[g - 1][1] - 1
        hi = GROUPS[g][1] - 1
        return [u for u, (rs, cs, _) in enumerate(UNITS) if lo < max(rs, cs) <= hi]

    # ---- emission: fast path (slot 0), later groups pipelined behind waves
    for ti in range(2):
        dma_group(ti, 0)
    for ti in range(2):
        sumsq_group(ti, 0, nc.vector if ti == 0 else nc.gpsimd)
        rnorm_group(ti, 0)
        apply_transpose_group(ti, 0)
    for ti in range(2):
        dma_group(ti, 1)
        sumsq_group(ti, 1, nc.vector if ti == 0 else nc.gpsimd)
    for u in wave(0):
        for ti in range(2):
            emit_unit(ti, u, split=True)
    for ti in range(2):
        rnorm_group(ti, 1)
        apply_transpose_group(ti, 1)
    for ti in range(2):
        dma_group(ti, 2)
        sumsq_group(ti, 2, nc.gpsimd if ti == 0 else nc.vector)
    for u in wave(1):
        for ti in range(2):
            emit_unit(ti, u)
    for ti in range(2):
        rnorm_group(ti, 2)
        apply_transpose_group(ti, 2)
    for ti in range(2):
        dma_group(ti, 3)
        sumsq_group(ti, 3, nc.gpsimd if ti == 0 else nc.vector)
    for u in wave(2):
        for ti in range(2):
            emit_unit(ti, u)
    for ti in range(2):
        rnorm_group(ti, 3)
        apply_transpose_group(ti, 3)

    # ---- align term from fp32 groups 0,1 (slots 0,1 = all N rows once) ----
    for g in range(2):
        qn = scratch.tile([128, CH, D], f32, tag="align_q")
        kn = scratch.tile([128, CH, D], f32, tag="align_k")
        sl = slice(CH * g, CH * (g + 1))
        rq = rns[0][:, sl, None].to_broadcast((128, CH, D))
        rk = rns[1][:, sl, None].to_broadcast((128, CH, D))
        nc.vector.tensor_tensor(qn[:], natf[0][g][:], rq, ALU.mult)
        nc.vector.tensor_tensor(kn[:], natf[1][g][:], rk, ALU.mult)
        nc.vector.tensor_tensor(qn[:], qn[:], kn[:], ALU.subtract)
        nc.gpsimd.tensor_tensor(qn[:], qn[:], qn[:], ALU.mult)
        nc.vector.tensor_reduce(acc_align[:, sl], qn[:], mybir.AxisListType.X, ALU.add)

    for u in wave(3):
        for ti in range(2):
            emit_unit(ti, u)

    # ---- write accumulators out
    nc.sync.dma_start(out_dram[:, 0:NACC], accs[0][:])
    nc.sync.dma_start(out_dram[:, NACC : 2 * NACC], accs[1][:])
    nc.sync.dma_start(out_dram[:, 2 * NACC : 2 * NACC + 8], acc_align[:])


@functools.lru_cache(maxsize=1)
def _build():
    from contextlib import ExitStack

    _apply_tile_exit_patch()
    nc = bacc.Bacc("TRN2", target_bir_lowering=False, debug=False, num_devices=NCORES)
    f32 = mybir.dt.float32
    qg = nc.dram_tensor("qg", [GROWS, D], f32, kind="ExternalInput")
    kg = nc.dram_tensor("kg", [GROWS, D], f32, kind="ExternalInput")
    out = nc.dram_tensor("out", [128, ACC_COLS], f32, kind="ExternalOutput")
    with tile.TileContext(nc) as tc, ExitStack() as ctx:
        _emit(nc, tc, ctx, (qg.ap(), kg.ap()), out.ap())
    nc.compile()
    return nc


def _gather(x: np.ndarray, c: int) -> np.ndarray:
    return np.ascontiguousarray(
        np.concatenate([x[BLK * b : BLK * (b + 1)] for b in _core_blocks(c)])
    )


def run_device(q: np.ndarray, k: np.ndarray, **run_kwargs):
    """Compile + run on the 8 cores; returns BassKernelResults."""
    from concourse.bass_utils import run_bass_kernel_spmd

    nc = _build()
    in_maps = [{"qg": _gather(q, c), "kg": _gather(k, c)} for c in range(NCORES)]
    return run_bass_kernel_spmd(nc, in_maps, core_ids=list(range(NCORES)), **run_kwargs)


def reduce_outputs(outs: list) -> np.float32:
    """Host-side gather/unshard: fold per-core accumulators into the scalar."""
    npairs = N * (N - 1) / 2.0
    terms = []
    for ti in range(2):
        off = 0.0
        diag = 0.0
        for c in range(NCORES):
            sums = outs[c]["out"][:, ti * NACC : (ti + 1) * NACC].astype(np.float64).sum(axis=0)
            for u, (_, _, is_diag) in enumerate(UNITS):
                if is_diag:
                    diag += sums[u]
                else:
                    off += sums[u]
            diag += sums[NU : NU + 4].sum()  # split unit-0 pieces (diag unit)
        upper = off + (diag - N) / 2.0
        terms.append(np.log(upper / npairs))
    align = (
        sum(
            outs[c]["out"][:, 2 * NACC : 2 * NACC + 8].astype(np.float64).sum()
            for c in range(NCORES)
        )
        / N
    )
    return np.float32(align + (terms[0] + terms[1]) / 2.0)


def kernel(q: np.ndarray, k: np.ndarray) -> np.ndarray:
    res = run_device(q, k)
    return np.asarray(reduce_outputs(res.results), dtype=np.float32)
